# revision 22
# baseline (speedup 1.0000x reference)
"""DeepSeekMoE block on 8 Trainium2 NeuronCores.

Sharding: expert-parallel — core e owns expert e's FFN (up_w[e]/down_w[e]);
tokens are dispatched to expert cores by host-side top-2 gating (the gate
matmul is 0.03% of total FLOPs).  The shared expert is token-parallel:
core e also runs the shared FFN for tokens [e*256, (e+1)*256).

Device kernel per core (SPMD), shared-expert work interleaved into the
routed loops so DMA demand is flat (~190 GB/s) and the PE never idles at a
phase boundary:
  up phase, per i-chunk ic:  hact_r[ic] = gelu(up_w[e].T @ xT + up_b[e])
                             hact_s[ic] = gelu(sw_up.T @ xsT + sb_up)
  down phase, per h-chunk:   eoT[hb] = 0.1*(down_w[e] @ hact_r + down_b[e])
                             soT[hb] = 0.1*(sw_down @ hact_s + sb_down)
Token blocks are (272,272) (fits one PSUM bank each; no thin-tail matmul).
Weights stream on two HWDGE rings: up-weights on the sync ring, down-weights
prefetched on the scalar ring, outputs on the (then idle) sync ring.

Host: gating/top-k (fp64 scores, fp32 combine weights), scatter-add of the
two expert contributions per token + shared path, row max-abs normalize.
"""
import sys
sys.path.insert(0, '/opt/trn_rl_repo')
import numpy as np
from contextlib import ExitStack

H = 1024
I = 4096
E = 8
TOPK = 2
B, S = 2, 1024
T = B * S            # 2048 tokens
CAP = 544            # routed-token capacity per expert core (max count is 542)
TS = T // E          # shared-expert tokens per core = 256
HC = H // 128        # 8 h-chunks
IC = I // 128        # 32 i-chunks
BLK_R = (272, 272)   # routed token blocks (each fits a 2KB PSUM bank)
SHARED_LAG = 5       # defer shared-up blocks so x DMAs win the early bandwidth

_COMPILED = {}


def _build_nc():
    from concourse import bacc, tile, mybir

    F32 = mybir.dt.float32
    CDT = mybir.dt.bfloat16
    GELU = mybir.ActivationFunctionType.Gelu
    IDENT = mybir.ActivationFunctionType.Identity

    nc = bacc.Bacc("TRN2", target_bir_lowering=False, debug=False, num_devices=E)

    xT_d = nc.dram_tensor("xT", [128, HC * CAP], CDT, kind="ExternalInput")
    xsT_d = nc.dram_tensor("xsT", [128, HC * TS], CDT, kind="ExternalInput")
    upw_d = nc.dram_tensor("upw", [128, IC * HC * 128], CDT, kind="ExternalInput")
    dnw_d = nc.dram_tensor("dnw", [128, HC * IC * 128], CDT, kind="ExternalInput")
    supw_d = nc.dram_tensor("supw", [128, IC * HC * 128], CDT, kind="ExternalInput")
    sdnw_d = nc.dram_tensor("sdnw", [128, HC * IC * 128], CDT, kind="ExternalInput")
    upb_d = nc.dram_tensor("upb", [128, IC], F32, kind="ExternalInput")
    supb_d = nc.dram_tensor("supb", [128, IC], F32, kind="ExternalInput")
    dnb_d = nc.dram_tensor("dnb", [128, HC], F32, kind="ExternalInput")
    sdnb_d = nc.dram_tensor("sdnb", [128, HC], F32, kind="ExternalInput")
    eoT_d = nc.dram_tensor("eoT", [HC, 128, CAP], F32, kind="ExternalOutput")
    soT_d = nc.dram_tensor("soT", [HC, 128, TS], F32, kind="ExternalOutput")

    with tile.TileContext(nc) as tc, ExitStack() as ctx:
        pool = ctx.enter_context(tc.tile_pool(name="sbuf", bufs=1))
        wpool = ctx.enter_context(tc.tile_pool(name="wstream", bufs=14))
        dwpool = ctx.enter_context(tc.tile_pool(name="dwstream", bufs=6))
        opool = ctx.enter_context(tc.tile_pool(name="outs", bufs=6))
        rps = ctx.enter_context(tc.tile_pool(name="rps", bufs=4, space="PSUM"))
        sps = ctx.enter_context(tc.tile_pool(name="sps", bufs=3, space="PSUM"))
        wps = ctx.enter_context(tc.tile_pool(name="wps", bufs=1, space="PSUM"))

        # --- PE warmup: a few dummy matmuls bridge the gap between the
        # framework prologue (~6us; no DMA can land earlier) and the first
        # real weight chunk, keeping the HAM activity window fed.
        warm_t = pool.tile([128, 512], CDT, tag="warm")
        nc.vector.memset(warm_t[:], 0)
        warm_ps = wps.tile([128, 512], F32, tag="ps")
        for _ in range(4):
            nc.tensor.matmul(warm_ps[:], warm_t[:, :128], warm_t[:],
                             start=True, stop=True)

        # --- first up-weight chunk leads the sync ring so the real matmul
        # stream can start as soon as the scalar ring delivers xT's first half
        uw0 = wpool.tile([128, HC * 128], CDT, tag="w")
        nc.sync.dma_start(uw0[:, :4 * 128], upw_d.ap()[:, :4 * 128])
        nc.sync.dma_start(uw0[:, 4 * 128:HC * 128], upw_d.ap()[:, 4 * 128:HC * 128])

        # --- resident activations + biases, split across BOTH HWDGE rings so
        # the full xT (needed within ~2us of the first matmul group) arrives
        # in parallel with the first up-weight chunks
        xT_t = pool.tile([128, HC * CAP], CDT, tag="xT")
        nc.scalar.dma_start(xT_t[:, 0:4 * CAP], xT_d.ap()[:, 0:4 * CAP])
        nc.sync.dma_start(xT_t[:, 4 * CAP:HC * CAP],
                          xT_d.ap()[:, 4 * CAP:HC * CAP])
        upb_t = pool.tile([128, IC], F32, tag="upb")
        nc.scalar.dma_start(upb_t[:], upb_d.ap()[:])
        supb_t = pool.tile([128, IC], F32, tag="supb")
        nc.scalar.dma_start(supb_t[:], supb_d.ap()[:])
        xsT_t = pool.tile([128, HC * TS], CDT, tag="xsT")
        nc.scalar.dma_start(xsT_t[:], xsT_d.ap()[:])
        dnb_t = pool.tile([128, HC], F32, tag="dnb")
        nc.scalar.dma_start(dnb_t[:], dnb_d.ap()[:])
        sdnb_t = pool.tile([128, HC], F32, tag="sdnb")
        nc.scalar.dma_start(sdnb_t[:], sdnb_d.ap()[:])

        # resident gelu activations (single tiles; pooled tiles bloat the
        # TileContext teardown with per-slot semaphore events)
        hact_r = pool.tile([128, IC * CAP], CDT, tag="hact_r")
        hact_s = pool.tile([128, IC * TS], CDT, tag="hact_s")

        # --- down-weight prefetch on the scalar HWDGE ring ---
        # Only fresh pool slots may be triggered before the down loop (a
        # slot-reuse wait here would wedge the scalar engine FIFO).
        dwtiles = {}

        def dw_fetch(k):
            """k even -> dnw chunk k//2, k odd -> sdnw chunk k//2."""
            hb = k // 2
            src = dnw_d if k % 2 == 0 else sdnw_d
            t = dwpool.tile([128, IC * 128], CDT, tag="dw")
            nc.scalar.dma_start(t[:], src.ap()[:, hb * IC * 128:(hb + 1) * IC * 128])
            dwtiles[k] = t

        # (first fetches are staggered into the up loop below so they don't
        # steal HBM bandwidth from the latency-critical up-weight stream)

        # --- merged UP phase ---
        # While the DMA rings ramp (first ~40us) alternate weight chunks
        # across BOTH rings; straggler cores otherwise stall on single-ring
        # supply jitter.  After ic 15 the scalar ring is needed for the
        # down-weight prefetch, so the stream returns to the sync ring.
        wcount = [0]

        def w_ring(ic):
            wcount[0] += 1
            return nc.scalar if (ic < 15 and wcount[0] % 2 == 1) else nc.sync

        def up_shared(ic):
            sw = wpool.tile([128, HC * 128], CDT, tag="w")
            w_ring(ic).dma_start(sw[:], supw_d.ap()[:, ic * HC * 128:(ic + 1) * HC * 128])
            ps = sps.tile([128, TS], F32, tag="ps")
            for hc in range(HC):
                nc.tensor.matmul(
                    ps[:], sw[:, hc * 128:(hc + 1) * 128],
                    xsT_t[:, hc * TS:(hc + 1) * TS],
                    start=(hc == 0), stop=(hc == HC - 1))
            nc.scalar.activation(hact_s[:, ic * TS:(ic + 1) * TS], ps[:], GELU,
                                 bias=supb_t[:, ic:ic + 1])

        for ic in range(IC):
            if ic == 0:
                uw = uw0   # fetched ahead of xT on the sync ring
            else:
                uw = wpool.tile([128, HC * 128], CDT, tag="w")
                w_ring(ic).dma_start(uw[:], upw_d.ap()[:, ic * HC * 128:(ic + 1) * HC * 128])
            t0 = 0
            for nb in BLK_R:
                ps = rps.tile([128, nb], F32, tag="ps")
                for hc in range(HC):
                    nc.tensor.matmul(
                        ps[:], uw[:, hc * 128:(hc + 1) * 128],
                        xT_t[:, hc * CAP + t0: hc * CAP + t0 + nb],
                        start=(hc == 0), stop=(hc == HC - 1))
                nc.scalar.activation(hact_r[:, ic * CAP + t0:ic * CAP + t0 + nb],
                                     ps[:], GELU, bias=upb_t[:, ic:ic + 1])
                t0 += nb

            if ic >= SHARED_LAG:
                up_shared(ic - SHARED_LAG)
            # down-weight prefetch in the back third of the up phase, after
            # the up-weight stream has built its own lookahead (earlier
            # fetches contend for HBM bandwidth and stall the PE)
            if ic >= 16 and (ic - 16) % 3 == 0 and (ic - 16) // 3 < 6:
                dw_fetch((ic - 16) // 3)

        for ic in range(IC - SHARED_LAG, IC):
            up_shared(ic)

        # --- merged DOWN phase ---
        for hb in range(HC):
            k = 2 * hb
            if k not in dwtiles:
                dw_fetch(k)
            dw = dwtiles[k]
            t0 = 0
            for nb in BLK_R:
                ps = rps.tile([128, nb], F32, tag="ps")
                for icc in range(IC):
                    nc.tensor.matmul(
                        ps[:], dw[:, icc * 128:(icc + 1) * 128],
                        hact_r[:, icc * CAP + t0:icc * CAP + t0 + nb],
                        start=(icc == 0), stop=(icc == IC - 1))
                ot = opool.tile([128, nb], F32, tag="out")
                nc.scalar.activation(ot[:], ps[:], IDENT,
                                     bias=dnb_t[:, hb:hb + 1], scale=0.1)
                nc.sync.dma_start(eoT_d.ap()[hb, :, t0:t0 + nb], ot[:])
                t0 += nb

            # fetch 3 iterations ahead so the transfer (~3us) completes well
            # before its consumer (slots freed by hb-3 are available; the
            # trigger's slot-reuse wait clears with this hb's routed MMs)
            for ka in (2 * hb + 6, 2 * hb + 7):
                if ka < 2 * HC and ka not in dwtiles:
                    dw_fetch(ka)

            k = 2 * hb + 1
            if k not in dwtiles:
                dw_fetch(k)
            sdw = dwtiles[k]
            ps = sps.tile([128, TS], F32, tag="ps")
            for icc in range(IC):
                nc.tensor.matmul(
                    ps[:], sdw[:, icc * 128:(icc + 1) * 128],
                    hact_s[:, icc * TS:(icc + 1) * TS],
                    start=(icc == 0), stop=(icc == IC - 1))
            ot = opool.tile([128, TS], F32, tag="out")
            nc.scalar.activation(ot[:], ps[:], IDENT,
                                 bias=sdnb_t[:, hb:hb + 1], scale=0.1)
            nc.sync.dma_start(soT_d.ap()[hb, :, :], ot[:])

    nc.compile()
    return nc


def _get_compiled():
    if "nc" not in _COMPILED:
        _COMPILED["nc"] = _build_nc()
    return _COMPILED["nc"]


def _np_cdt():
    import ml_dtypes
    return np.dtype(ml_dtypes.bfloat16)


def _pack_weight(w):
    """[K, N] -> [128, (N/128 chunks) x (K/128 subtiles) x 128] stream layout."""
    kdim, ndim = w.shape
    kc, nchunk = kdim // 128, ndim // 128
    return np.ascontiguousarray(
        w.reshape(kc, 128, nchunk, 128).transpose(1, 2, 0, 3)
    ).reshape(128, nchunk * kc * 128).astype(_np_cdt())


def _pack_tokens(xsel, cap):
    """[n, H] tokens -> [128, HC*cap] transposed h-chunked layout, zero pad."""
    n = xsel.shape[0]
    arr = np.zeros((128, HC, cap), np.float32)
    if n:
        arr[:, :, :n] = xsel.T.reshape(HC, 128, n).transpose(1, 0, 2)
    return np.ascontiguousarray(arr).reshape(128, HC * cap).astype(_np_cdt())


def _pack_bias(b, scale=1.0):
    """[N] -> [128, N/128] per-partition layout."""
    return np.ascontiguousarray(
        (np.asarray(b, np.float32) * scale).reshape(-1, 128).T.astype(np.float32))


def kernel(x, gate_w, bias, up_w, up_b, down_w, down_b,
           sw_up, sb_up, sw_down, sb_down):
    from concourse.bass_utils import run_bass_kernel_spmd

    x = np.asarray(x, np.float32)
    xf = x.reshape(T, H)

    # ---- host gating (fp64 scores for a stable top-k, fp32 combine weights)
    z64 = xf.astype(np.float64) @ np.asarray(gate_w, np.float64) \
        + np.asarray(bias, np.float64)
    scores64 = 1.0 / (1.0 + np.exp(-z64))
    top_idx = np.argsort(-scores64, axis=-1, kind="stable")[:, :TOPK]
    tsc = scores64[np.arange(T)[:, None], top_idx].astype(np.float32)
    wts = tsc / (tsc.sum(-1, keepdims=True) + np.float32(1e-6))   # [T, 2]

    # ---- token dispatch
    tok_lists = [np.where((top_idx == e).any(-1))[0] for e in range(E)]
    for e, tl in enumerate(tok_lists):
        if len(tl) > CAP:
            raise RuntimeError(f"expert {e} overflow: {len(tl)} > CAP={CAP}")

    supw = _pack_weight(np.asarray(sw_up, np.float32))
    sdnw = _pack_weight(np.asarray(sw_down, np.float32))
    supb = _pack_bias(sb_up)
    sdnb = _pack_bias(sb_down, scale=0.1)

    in_maps = []
    for e in range(E):
        in_maps.append({
            "xT": _pack_tokens(xf[tok_lists[e]], CAP),
            "xsT": _pack_tokens(xf[e * TS:(e + 1) * TS], TS),
            "upw": _pack_weight(np.asarray(up_w[e], np.float32)),
            "dnw": _pack_weight(np.asarray(down_w[e], np.float32)),
            "supw": supw,
            "sdnw": sdnw,
            "upb": _pack_bias(up_b[e]),
            "supb": supb,
            "dnb": _pack_bias(down_b[e], scale=0.1),
            "sdnb": sdnb,
        })

    nc = _get_compiled()
    res = run_bass_kernel_spmd(nc, in_maps, list(range(E)))

    # ---- host combine: scatter-add expert outputs, add shared, normalize
    out = np.zeros((T, H), np.float32)
    for e in range(E):
        soT = np.asarray(res.results[e]["soT"], np.float32)   # [HC, 128, TS]
        out[e * TS:(e + 1) * TS] = soT.reshape(H, TS).T
    for e in range(E):
        tl = tok_lists[e]
        if len(tl) == 0:
            continue
        eoT = np.asarray(res.results[e]["eoT"], np.float32)   # [HC, 128, CAP]
        eo = eoT.reshape(H, CAP)[:, :len(tl)].T               # [n, H]
        we = np.where(top_idx[tl, 0] == e, wts[tl, 0], wts[tl, 1]).astype(np.float32)
        out[tl] += we[:, None] * eo

    out /= (np.abs(out).max(-1, keepdims=True) + np.float32(1e-6))
    return out.reshape(B, S, H)


# revision 25
# speedup vs baseline: 1.0322x; 1.0322x over previous
"""DeepSeekMoE block on 8 Trainium2 NeuronCores.

Sharding: expert-parallel — core e owns expert e's FFN (up_w[e]/down_w[e]);
tokens are dispatched to expert cores by host-side top-2 gating (the gate
matmul is 0.03% of total FLOPs).  The shared expert is token-parallel:
core e also runs the shared FFN for tokens [e*256, (e+1)*256).

Device kernel per core (SPMD), shared-expert work interleaved into the
routed loops so DMA demand is flat (~190 GB/s) and the PE never idles at a
phase boundary:
  up phase, per i-chunk ic:  hact_r[ic] = gelu(up_w[e].T @ xT + up_b[e])
                             hact_s[ic] = gelu(sw_up.T @ xsT + sb_up)
  down phase, per h-chunk:   eoT[hb] = 0.1*(down_w[e] @ hact_r + down_b[e])
                             soT[hb] = 0.1*(sw_down @ hact_s + sb_down)
Token blocks are (272,272) (fits one PSUM bank each; no thin-tail matmul).
Weights stream on two HWDGE rings: up-weights on the sync ring, down-weights
prefetched on the scalar ring, outputs on the (then idle) sync ring.

Host: gating/top-k (fp64 scores, fp32 combine weights), scatter-add of the
two expert contributions per token + shared path, row max-abs normalize.
"""
import sys
sys.path.insert(0, '/opt/trn_rl_repo')
import numpy as np
from contextlib import ExitStack

H = 1024
I = 4096
E = 8
TOPK = 2
B, S = 2, 1024
T = B * S            # 2048 tokens
CAP = 544            # routed-token capacity per expert core (max count is 542)
TS = T // E          # shared-expert tokens per core = 256
HC = H // 128        # 8 h-chunks
IC = I // 128        # 32 i-chunks
BLK_R = (272, 272)   # routed token blocks (each fits a 2KB PSUM bank)
SHARED_LAG = 5       # defer shared-up blocks so x DMAs win the early bandwidth

_COMPILED = {}


def _build_nc():
    from concourse import bacc, tile, mybir

    F32 = mybir.dt.float32
    CDT = mybir.dt.bfloat16
    GELU = mybir.ActivationFunctionType.Gelu
    IDENT = mybir.ActivationFunctionType.Identity

    nc = bacc.Bacc("TRN2", target_bir_lowering=False, debug=False, num_devices=E)

    xT_d = nc.dram_tensor("xT", [128, HC * CAP], CDT, kind="ExternalInput")
    xsT_d = nc.dram_tensor("xsT", [128, HC * TS], CDT, kind="ExternalInput")
    upw_d = nc.dram_tensor("upw", [128, IC * HC * 128], CDT, kind="ExternalInput")
    dnw_d = nc.dram_tensor("dnw", [128, HC * IC * 128], CDT, kind="ExternalInput")
    supw_d = nc.dram_tensor("supw", [128, IC * HC * 128], CDT, kind="ExternalInput")
    sdnw_d = nc.dram_tensor("sdnw", [128, HC * IC * 128], CDT, kind="ExternalInput")
    upb_d = nc.dram_tensor("upb", [128, IC], F32, kind="ExternalInput")
    supb_d = nc.dram_tensor("supb", [128, IC], F32, kind="ExternalInput")
    dnb_d = nc.dram_tensor("dnb", [128, HC], F32, kind="ExternalInput")
    sdnb_d = nc.dram_tensor("sdnb", [128, HC], F32, kind="ExternalInput")
    eoT_d = nc.dram_tensor("eoT", [HC, 128, CAP], F32, kind="ExternalOutput")
    soT_d = nc.dram_tensor("soT", [HC, 128, TS], F32, kind="ExternalOutput")

    with tile.TileContext(nc) as tc, ExitStack() as ctx:
        pool = ctx.enter_context(tc.tile_pool(name="sbuf", bufs=1))
        wpool = ctx.enter_context(tc.tile_pool(name="wstream", bufs=14))
        dwpool = ctx.enter_context(tc.tile_pool(name="dwstream", bufs=6))
        opool = ctx.enter_context(tc.tile_pool(name="outs", bufs=6))
        rps = ctx.enter_context(tc.tile_pool(name="rps", bufs=4, space="PSUM"))
        sps = ctx.enter_context(tc.tile_pool(name="sps", bufs=3, space="PSUM"))
        wps = ctx.enter_context(tc.tile_pool(name="wps", bufs=1, space="PSUM"))

        # --- PE warmup: a few dummy matmuls bridge the gap between the
        # framework prologue (~6us; no DMA can land earlier) and the first
        # real weight chunk, keeping the HAM activity window fed.
        warm_t = pool.tile([128, 512], CDT, tag="warm")
        nc.vector.memset(warm_t[:], 0)
        warm_ps = wps.tile([128, 512], F32, tag="ps")
        for _ in range(6):
            nc.tensor.matmul(warm_ps[:], warm_t[:, :128], warm_t[:],
                             start=True, stop=True)

        # --- first up-weight chunk leads the sync ring so the real matmul
        # stream can start as soon as the scalar ring delivers xT's first half
        uw0 = wpool.tile([128, HC * 128], CDT, tag="w")
        nc.sync.dma_start(uw0[:, :4 * 128], upw_d.ap()[:, :4 * 128])
        nc.sync.dma_start(uw0[:, 4 * 128:HC * 128], upw_d.ap()[:, 4 * 128:HC * 128])

        # --- resident activations + biases, split across BOTH HWDGE rings so
        # the full xT (needed within ~2us of the first matmul group) arrives
        # in parallel with the first up-weight chunks
        xT_t = pool.tile([128, HC * CAP], CDT, tag="xT")
        nc.scalar.dma_start(xT_t[:, 0:4 * CAP], xT_d.ap()[:, 0:4 * CAP])
        nc.sync.dma_start(xT_t[:, 4 * CAP:HC * CAP],
                          xT_d.ap()[:, 4 * CAP:HC * CAP])
        upb_t = pool.tile([128, IC], F32, tag="upb")
        nc.scalar.dma_start(upb_t[:], upb_d.ap()[:])
        supb_t = pool.tile([128, IC], F32, tag="supb")
        nc.scalar.dma_start(supb_t[:], supb_d.ap()[:])
        xsT_t = pool.tile([128, HC * TS], CDT, tag="xsT")
        nc.scalar.dma_start(xsT_t[:], xsT_d.ap()[:])
        dnb_t = pool.tile([128, HC], F32, tag="dnb")
        nc.scalar.dma_start(dnb_t[:], dnb_d.ap()[:])
        sdnb_t = pool.tile([128, HC], F32, tag="sdnb")
        nc.scalar.dma_start(sdnb_t[:], sdnb_d.ap()[:])

        # resident gelu activations (single tiles; pooled tiles bloat the
        # TileContext teardown with per-slot semaphore events)
        hact_r = pool.tile([128, IC * CAP], CDT, tag="hact_r")
        hact_s = pool.tile([128, IC * TS], CDT, tag="hact_s")

        # --- down-weight prefetch on the scalar HWDGE ring ---
        # Only fresh pool slots may be triggered before the down loop (a
        # slot-reuse wait here would wedge the scalar engine FIFO).
        dwtiles = {}

        def dw_fetch(k):
            """k even -> dnw chunk k//2, k odd -> sdnw chunk k//2."""
            hb = k // 2
            src = dnw_d if k % 2 == 0 else sdnw_d
            t = dwpool.tile([128, IC * 128], CDT, tag="dw")
            nc.scalar.dma_start(t[:], src.ap()[:, hb * IC * 128:(hb + 1) * IC * 128])
            dwtiles[k] = t

        # (first fetches are staggered into the up loop below so they don't
        # steal HBM bandwidth from the latency-critical up-weight stream)

        # --- merged UP phase ---
        def up_shared(ic):
            sw = wpool.tile([128, HC * 128], CDT, tag="w")
            nc.sync.dma_start(sw[:], supw_d.ap()[:, ic * HC * 128:(ic + 1) * HC * 128])
            ps = sps.tile([128, TS], F32, tag="ps")
            for hc in range(HC):
                nc.tensor.matmul(
                    ps[:], sw[:, hc * 128:(hc + 1) * 128],
                    xsT_t[:, hc * TS:(hc + 1) * TS],
                    start=(hc == 0), stop=(hc == HC - 1))
            nc.scalar.activation(hact_s[:, ic * TS:(ic + 1) * TS], ps[:], GELU,
                                 bias=supb_t[:, ic:ic + 1])

        for ic in range(IC):
            if ic == 0:
                uw = uw0   # fetched ahead of xT on the sync ring
            else:
                uw = wpool.tile([128, HC * 128], CDT, tag="w")
                nc.sync.dma_start(uw[:], upw_d.ap()[:, ic * HC * 128:(ic + 1) * HC * 128])
            t0 = 0
            for nb in BLK_R:
                ps = rps.tile([128, nb], F32, tag="ps")
                for hc in range(HC):
                    nc.tensor.matmul(
                        ps[:], uw[:, hc * 128:(hc + 1) * 128],
                        xT_t[:, hc * CAP + t0: hc * CAP + t0 + nb],
                        start=(hc == 0), stop=(hc == HC - 1))
                nc.scalar.activation(hact_r[:, ic * CAP + t0:ic * CAP + t0 + nb],
                                     ps[:], GELU, bias=upb_t[:, ic:ic + 1])
                t0 += nb

            if ic >= SHARED_LAG:
                up_shared(ic - SHARED_LAG)
            # down-weight prefetch in the back third of the up phase, after
            # the up-weight stream has built its own lookahead (earlier
            # fetches contend for HBM bandwidth and stall the PE)
            if ic >= 16 and (ic - 16) % 3 == 0 and (ic - 16) // 3 < 6:
                dw_fetch((ic - 16) // 3)

        for ic in range(IC - SHARED_LAG, IC):
            up_shared(ic)

        # --- merged DOWN phase ---
        for hb in range(HC):
            k = 2 * hb
            if k not in dwtiles:
                dw_fetch(k)
            dw = dwtiles[k]
            t0 = 0
            for nb in BLK_R:
                ps = rps.tile([128, nb], F32, tag="ps")
                for icc in range(IC):
                    nc.tensor.matmul(
                        ps[:], dw[:, icc * 128:(icc + 1) * 128],
                        hact_r[:, icc * CAP + t0:icc * CAP + t0 + nb],
                        start=(icc == 0), stop=(icc == IC - 1))
                ot = opool.tile([128, nb], F32, tag="out")
                nc.scalar.activation(ot[:], ps[:], IDENT,
                                     bias=dnb_t[:, hb:hb + 1], scale=0.1)
                nc.sync.dma_start(eoT_d.ap()[hb, :, t0:t0 + nb], ot[:])
                t0 += nb

            # fetch 3 iterations ahead so the transfer (~3us) completes well
            # before its consumer (slots freed by hb-3 are available; the
            # trigger's slot-reuse wait clears with this hb's routed MMs)
            for ka in (2 * hb + 6, 2 * hb + 7):
                if ka < 2 * HC and ka not in dwtiles:
                    dw_fetch(ka)

            k = 2 * hb + 1
            if k not in dwtiles:
                dw_fetch(k)
            sdw = dwtiles[k]
            ps = sps.tile([128, TS], F32, tag="ps")
            for icc in range(IC):
                nc.tensor.matmul(
                    ps[:], sdw[:, icc * 128:(icc + 1) * 128],
                    hact_s[:, icc * TS:(icc + 1) * TS],
                    start=(icc == 0), stop=(icc == IC - 1))
            ot = opool.tile([128, TS], F32, tag="out")
            nc.scalar.activation(ot[:], ps[:], IDENT,
                                 bias=sdnb_t[:, hb:hb + 1], scale=0.1)
            nc.sync.dma_start(soT_d.ap()[hb, :, :], ot[:])

    nc.compile()
    return nc


def _get_compiled():
    if "nc" not in _COMPILED:
        _COMPILED["nc"] = _build_nc()
    return _COMPILED["nc"]


def _np_cdt():
    import ml_dtypes
    return np.dtype(ml_dtypes.bfloat16)


def _pack_weight(w):
    """[K, N] -> [128, (N/128 chunks) x (K/128 subtiles) x 128] stream layout."""
    kdim, ndim = w.shape
    kc, nchunk = kdim // 128, ndim // 128
    return np.ascontiguousarray(
        w.reshape(kc, 128, nchunk, 128).transpose(1, 2, 0, 3)
    ).reshape(128, nchunk * kc * 128).astype(_np_cdt())


def _pack_tokens(xsel, cap):
    """[n, H] tokens -> [128, HC*cap] transposed h-chunked layout, zero pad."""
    n = xsel.shape[0]
    arr = np.zeros((128, HC, cap), np.float32)
    if n:
        arr[:, :, :n] = xsel.T.reshape(HC, 128, n).transpose(1, 0, 2)
    return np.ascontiguousarray(arr).reshape(128, HC * cap).astype(_np_cdt())


def _pack_bias(b, scale=1.0):
    """[N] -> [128, N/128] per-partition layout."""
    return np.ascontiguousarray(
        (np.asarray(b, np.float32) * scale).reshape(-1, 128).T.astype(np.float32))


def kernel(x, gate_w, bias, up_w, up_b, down_w, down_b,
           sw_up, sb_up, sw_down, sb_down):
    from concourse.bass_utils import run_bass_kernel_spmd

    x = np.asarray(x, np.float32)
    xf = x.reshape(T, H)

    # ---- host gating (fp64 scores for a stable top-k, fp32 combine weights)
    z64 = xf.astype(np.float64) @ np.asarray(gate_w, np.float64) \
        + np.asarray(bias, np.float64)
    scores64 = 1.0 / (1.0 + np.exp(-z64))
    top_idx = np.argsort(-scores64, axis=-1, kind="stable")[:, :TOPK]
    tsc = scores64[np.arange(T)[:, None], top_idx].astype(np.float32)
    wts = tsc / (tsc.sum(-1, keepdims=True) + np.float32(1e-6))   # [T, 2]

    # ---- token dispatch
    tok_lists = [np.where((top_idx == e).any(-1))[0] for e in range(E)]
    for e, tl in enumerate(tok_lists):
        if len(tl) > CAP:
            raise RuntimeError(f"expert {e} overflow: {len(tl)} > CAP={CAP}")

    supw = _pack_weight(np.asarray(sw_up, np.float32))
    sdnw = _pack_weight(np.asarray(sw_down, np.float32))
    supb = _pack_bias(sb_up)
    sdnb = _pack_bias(sb_down, scale=0.1)

    in_maps = []
    for e in range(E):
        in_maps.append({
            "xT": _pack_tokens(xf[tok_lists[e]], CAP),
            "xsT": _pack_tokens(xf[e * TS:(e + 1) * TS], TS),
            "upw": _pack_weight(np.asarray(up_w[e], np.float32)),
            "dnw": _pack_weight(np.asarray(down_w[e], np.float32)),
            "supw": supw,
            "sdnw": sdnw,
            "upb": _pack_bias(up_b[e]),
            "supb": supb,
            "dnb": _pack_bias(down_b[e], scale=0.1),
            "sdnb": sdnb,
        })

    nc = _get_compiled()
    res = run_bass_kernel_spmd(nc, in_maps, list(range(E)))

    # ---- host combine: scatter-add expert outputs, add shared, normalize
    out = np.zeros((T, H), np.float32)
    for e in range(E):
        soT = np.asarray(res.results[e]["soT"], np.float32)   # [HC, 128, TS]
        out[e * TS:(e + 1) * TS] = soT.reshape(H, TS).T
    for e in range(E):
        tl = tok_lists[e]
        if len(tl) == 0:
            continue
        eoT = np.asarray(res.results[e]["eoT"], np.float32)   # [HC, 128, CAP]
        eo = eoT.reshape(H, CAP)[:, :len(tl)].T               # [n, H]
        we = np.where(top_idx[tl, 0] == e, wts[tl, 0], wts[tl, 1]).astype(np.float32)
        out[tl] += we[:, None] * eo

    out /= (np.abs(out).max(-1, keepdims=True) + np.float32(1e-6))
    return out.reshape(B, S, H)
